# revision 11
# baseline (speedup 1.0000x reference)
"""Bidirectional Mamba block on 8 Trainium2 NeuronCores.

Sharding: core c -> (batch b = c//4, direction d = (c%4)//2, d_inner half h = c%2).
Each core runs an identical Bass/Tile program; all per-core differences are in the
input data (weights pre-sliced/transposed on host, bwd cores get time-flipped x).

Per-core pipeline (everything in [feature-partition, time-free] layout after an
on-device PE transpose of the layernormed input):
  LN -> transpose -> in_proj (xc full + z half) -> causal conv4 + silu ->
  xproj (dt/B/C) -> dt_proj + softplus -> dA=exp(delta*A) (ACT, per-partition
  scale) -> dBu (free-dim broadcast mult) -> tensor_tensor_scan over time per
  (d, n) lane -> C-contraction (mult + tree reduce over n) -> D skip + silu(z)
  gate -> out_proj partial.
Host sums the two d_inner-half partials, flips the bwd direction back, and adds
the residual.
"""

import numpy as np
import ml_dtypes

import concourse.bass as bass
import concourse.bacc as bacc
import concourse.tile as tile
from concourse import mybir
from concourse import bass_utils
from concourse.masks import make_identity

F32 = mybir.dt.float32
F32R = mybir.dt.float32r
BF16 = mybir.dt.bfloat16
AF = mybir.ActivationFunctionType
ALU = mybir.AluOpType

N_CORES = 8
L = 1024          # sequence length
DM = 768          # d_model
DI = 1536         # d_inner
DH = 768          # d_inner half per core
DT_RANK = 48
NS = 16           # d_state
DC = 4            # d_conv
TC = 512          # time chunk for the scan block
NT = L // TC
KM = DM // 128    # 6  k-tiles over d_model
DBH = DH // 128   # 6  d-blocks in my half
DBF = DI // 128   # 12 d-blocks full d_inner
NXZ = DI + DH     # 2304 in_proj output channels (xc full + z half)
EPS = 1e-5


def _bcast_part(ap2d, parts=128):
    """[1, F] row AP -> [parts, F] partition-broadcast AP (step 0)."""
    return bass.AP(tensor=ap2d.tensor, offset=ap2d.offset,
                   ap=[[0, parts]] + [list(e) for e in ap2d.ap[1:]])


def _free_repeat(ap2d, times):
    """[P, F] AP -> [P, times, F] with a step-0 middle free dim."""
    return bass.AP(tensor=ap2d.tensor, offset=ap2d.offset,
                   ap=[list(ap2d.ap[0]), [0, times]] + [list(e) for e in ap2d.ap[1:]])


def build_nc():
    nc = bacc.Bacc("TRN2", target_bir_lowering=False, debug=False,
                   num_devices=N_CORES)

    # ---- DRAM I/O ----
    xin = nc.dram_tensor("xin", (L, DM), F32, kind="ExternalInput")
    w_xz = nc.dram_tensor("w_xz", (DM, NXZ), BF16, kind="ExternalInput")
    b_xz = nc.dram_tensor("b_xz", (NXZ, 1), F32, kind="ExternalInput")
    w_cv = nc.dram_tensor("w_cv", (DI, DC), F32, kind="ExternalInput")
    b_cv = nc.dram_tensor("b_cv", (DI, 1), F32, kind="ExternalInput")
    w_xp = nc.dram_tensor("w_xp", (DI, 96), BF16, kind="ExternalInput")
    w_dt = nc.dram_tensor("w_dt", (DT_RANK, DH), BF16, kind="ExternalInput")
    b_dt = nc.dram_tensor("b_dt", (DH, 1), F32, kind="ExternalInput")
    a_h = nc.dram_tensor("a_h", (DH, NS), F32, kind="ExternalInput")
    d_h = nc.dram_tensor("d_h", (DH, 1), F32, kind="ExternalInput")
    w_out = nc.dram_tensor("w_out", (DH, DM), BF16, kind="ExternalInput")
    outp = nc.dram_tensor("outp", (DM, L), F32, kind="ExternalOutput")
    bc_dram = nc.dram_tensor("bc_scratch", (32, L), BF16, kind="Internal")

    with tile.TileContext(nc) as tc:
        with (
            tc.tile_pool(name="const", bufs=1) as cpool,
            tc.tile_pool(name="persist", bufs=1) as ppool,
            tc.tile_pool(name="psA", bufs=4, space="PSUM") as psA,
            tc.tile_pool(name="psT", bufs=2, space="PSUM") as psT,
        ):
            # ---- constants ----
            ident = cpool.tile([128, 128], BF16, name="ident")
            make_identity(nc, ident)
            eps_t = cpool.tile([128, 1], F32, name="eps_t")
            nc.vector.memset(eps_t, EPS)

            bxz_t = cpool.tile([128, NXZ // 128], F32, name="bxz_t")   # [128, 18]
            nc.sync.dma_start(out=bxz_t, in_=b_xz.ap().rearrange("(a p) o -> p (a o)", p=128))
            bcv_t = cpool.tile([128, DBF], F32, name="bcv_t")
            nc.sync.dma_start(out=bcv_t, in_=b_cv.ap().rearrange("(a p) o -> p (a o)", p=128))
            wcv_t = cpool.tile([128, DBF, DC], F32, name="wcv_t")
            nc.sync.dma_start(out=wcv_t, in_=w_cv.ap().rearrange("(a p) c -> p a c", p=128))
            bdt_t = cpool.tile([128, DBH], F32, name="bdt_t")
            nc.sync.dma_start(out=bdt_t, in_=b_dt.ap().rearrange("(a p) o -> p (a o)", p=128))
            a_t = cpool.tile([128, DBH, NS], F32, name="a_t")
            nc.sync.dma_start(out=a_t, in_=a_h.ap().rearrange("(a p) n -> p a n", p=128))
            d_t = cpool.tile([128, DBH], F32, name="d_t")
            nc.sync.dma_start(out=d_t, in_=d_h.ap().rearrange("(a p) o -> p (a o)", p=128))

            # persistent activation tiles (live until the end)
            zs = [ppool.tile([128, L], BF16, name=f"zs{j}") for j in range(DBH)]
            xcb = [ppool.tile([128, L], BF16, name=f"xcb{j}") for j in range(DBH)]
            # dbc is bf16: it feeds the dt_proj matmul directly
            delta = [ppool.tile([128, L], BF16, name=f"dl{j}") for j in range(DBH)]
            y_acc = [ppool.tile([128, L], BF16, name=f"ya{j}") for j in range(DBH)]
            dbc = ppool.tile([96, L], BF16, name="dbc")
            bcb = ppool.tile([32, L], BF16, name="bcb")
            hcol = [ppool.tile([128, NS], BF16, name=f"hc{j}") for j in range(DBH)]

            with tc.tile_pool(name="xcrp", bufs=1) as xcrp:
                xcr = [xcrp.tile([128, L], F32, name=f"xcr{j}") for j in range(DBF)]

                with tc.tile_pool(name="x0Tp", bufs=1) as x0Tp:
                    x0T = [x0Tp.tile([128, L], BF16, name=f"x0T{j}") for j in range(KM)]

                    # ---- stage 0: load x, layernorm (rows = time) ----
                    with tc.tile_pool(name="ln", bufs=1) as lnp:
                        x0 = []
                        for i in range(L // 128):
                            xt = lnp.tile([128, DM], F32, name=f"xt{i}")
                            nc.sync.dma_start(out=xt, in_=xin.ap()[i * 128:(i + 1) * 128, :])
                            st = lnp.tile([128, 3, 6], F32, tag="st", name="st")
                            xg = xt[:].rearrange("p (s f) -> p s f", s=3)
                            for s in range(3):
                                nc.vector.bn_stats(out=st[:, s, :], in_=xg[:, s, :])
                            mv = lnp.tile([128, 2], F32, tag="mv", name="mv")
                            nc.vector.bn_aggr(out=mv, in_=st)
                            sd = lnp.tile([128, 1], F32, tag="sd", name="sd")
                            nc.scalar.activation(out=sd, in_=mv[:, 1:2], func=AF.Ln,
                                                 bias=eps_t[:, 0:1], scale=1.0)
                            rs = lnp.tile([128, 1], F32, tag="rs", name="rs")
                            nc.scalar.activation(out=rs, in_=sd, func=AF.Exp,
                                                 bias=0.0, scale=-0.5)
                            x0t = lnp.tile([128, DM], BF16, name=f"x0_{i}")
                            nc.vector.tensor_scalar(out=x0t, in0=xt, scalar1=mv[:, 0:1],
                                                    scalar2=rs[:, 0:1], op0=ALU.subtract,
                                                    op1=ALU.mult)
                            x0.append(x0t)

                        # ---- stage 1: transpose x0 -> x0T [DM, L] ----
                        for dj in range(KM):
                            for half in range(2):
                                pt = psT.tile([128, 512], BF16, tag="pt", name="pt")
                                for tt in range(4):
                                    ti = half * 4 + tt
                                    nc.tensor.transpose(
                                        out=pt[:, tt * 128:(tt + 1) * 128],
                                        in_=x0[ti][:, dj * 128:(dj + 1) * 128],
                                        identity=ident)
                                nc.any.tensor_copy(
                                    out=x0T[dj][:, half * 512:(half + 1) * 512], in_=pt)

                    # ---- stage 2: in_proj ----
                    with tc.tile_pool(name="wxzp", bufs=1) as wxzp:
                        wxz_t = [wxzp.tile([128, NXZ], BF16, name=f"wxz{k}") for k in range(KM)]
                        for k in range(KM):
                            nc.sync.dma_start(out=wxz_t[k], in_=w_xz.ap()[k * 128:(k + 1) * 128, :])
                        for mi in range(NXZ // 128):          # 18
                            for f in range(2):
                                pm = psA.tile([128, 512], F32, tag="ps", name="ps")
                                for k in range(KM):
                                    nc.tensor.matmul(
                                        out=pm,
                                        lhsT=wxz_t[k][:, mi * 128:(mi + 1) * 128],
                                        rhs=x0T[k][:, f * 512:(f + 1) * 512],
                                        start=(k == 0), stop=(k == KM - 1))
                                if mi < DBF:
                                    nc.scalar.activation(
                                        out=xcr[mi][:, f * 512:(f + 1) * 512], in_=pm,
                                        func=AF.Identity, bias=bxz_t[:, mi:mi + 1], scale=1.0)
                                else:
                                    nc.scalar.activation(
                                        out=zs[mi - DBF][:, f * 512:(f + 1) * 512], in_=pm,
                                        func=AF.Silu, bias=bxz_t[:, mi:mi + 1], scale=1.0)

                # ---- stage 3: causal conv4 + silu -> bf16 ----
                with tc.tile_pool(name="cv", bufs=2) as cvp:
                    xcp = [xcrp.tile([128, L], BF16, name=f"xcp{j}") for j in range(DBF - DBH)]
                    xcs = xcb + xcp
                    for j in range(DBF):
                        acc = cvp.tile([128, L], F32, tag="acc", name="acc")
                        nc.vector.tensor_scalar(out=acc, in0=xcr[j],
                                                scalar1=wcv_t[:, j, 0:1], scalar2=None,
                                                op0=ALU.mult)
                        for k in range(1, DC):
                            nc.vector.scalar_tensor_tensor(
                                out=acc[:, k:L], in0=xcr[j][:, 0:L - k],
                                scalar=wcv_t[:, j, k:k + 1], in1=acc[:, k:L],
                                op0=ALU.mult, op1=ALU.add)
                        nc.scalar.activation(out=xcs[j], in_=acc, func=AF.Silu,
                                             bias=bcv_t[:, j:j + 1], scale=1.0)

                    # ---- stage 4: xproj -> dbc [80, L] ----
                    wxp_t = [cvp.tile([128, 96], BF16, name=f"wxp{k}") for k in range(DBF)]
                    for k in range(DBF):
                        nc.sync.dma_start(out=wxp_t[k], in_=w_xp.ap()[k * 128:(k + 1) * 128, :])
                    for f in range(2):
                        pm128 = psA.tile([128, 512], F32, tag="ps", name="ps")
                        pm = pm128[0:96, :]
                        for k in range(DBF):
                            nc.tensor.matmul(
                                out=pm, lhsT=wxp_t[k][:],
                                rhs=xcs[k][:, f * 512:(f + 1) * 512],
                                start=(k == 0), stop=(k == DBF - 1))
                        nc.vector.tensor_copy(out=dbc[:, f * 512:(f + 1) * 512], in_=pm)

                    # ---- stage 5: dt_proj + softplus -> delta (bf16) ----
                    wdt_t = cvp.tile([DT_RANK, DH], BF16, name="wdt_t")
                    nc.sync.dma_start(out=wdt_t, in_=w_dt.ap())
                    for mj in range(DBH):
                        for f in range(2):
                            pm = psA.tile([128, 512], F32, tag="ps", name="ps")
                            nc.tensor.matmul(
                                out=pm,
                                lhsT=wdt_t[:, mj * 128:(mj + 1) * 128],
                                rhs=dbc[0:DT_RANK, f * 512:(f + 1) * 512],
                                start=True, stop=True)
                            et = cvp.tile([128, 512], F32, tag="spe", name="spe")
                            nc.scalar.activation(out=et, in_=pm, func=AF.Exp,
                                                 bias=bdt_t[:, mj:mj + 1], scale=1.0)
                            nc.scalar.activation(out=delta[mj][:, f * 512:(f + 1) * 512],
                                                 in_=et, func=AF.Ln,
                                                 bias=1.0, scale=1.0)
                    nc.vector.tensor_copy(out=bcb, in_=dbc[64:96, :])
                    nc.sync.dma_start(out=bc_dram.ap(), in_=bcb)

            # ---- stage 6/7: scan block ----
            with (
                tc.tile_pool(name="bc", bufs=1) as bcp,
                tc.tile_pool(name="dap", bufs=2) as dap,
                tc.tile_pool(name="sc", bufs=1) as scp,
            ):
                for t in range(NT):
                    tsl = slice(t * TC, (t + 1) * TC)
                    B_all = bcp.tile([128, NS * TC], BF16, tag="Ball", name="Ball")
                    C_all = bcp.tile([128, NS * TC], BF16, tag="Call", name="Call")
                    for n in range(NS):
                        nc.sync.dma_start(out=B_all[:, n * TC:(n + 1) * TC],
                                          in_=_bcast_part(bc_dram.ap()[n:n + 1, tsl]))
                        nc.sync.dma_start(out=C_all[:, n * TC:(n + 1) * TC],
                                          in_=_bcast_part(bc_dram.ap()[NS + n:NS + n + 1, tsl]))
                    for j in range(DBH):
                        da = dap.tile([128, NS * TC], BF16, tag="da", name="da")
                        for n in range(NS):
                            nc.scalar.activation(out=da[:, n * TC:(n + 1) * TC],
                                                 in_=delta[j][:, tsl], func=AF.Exp,
                                                 bias=0.0, scale=a_t[:, j, n:n + 1])
                        dx = scp.tile([128, TC], BF16, tag="dx", name="dx")
                        nc.vector.tensor_mul(out=dx, in0=delta[j][:, tsl],
                                             in1=xcb[j][:, tsl])
                        db = scp.tile([128, NS * TC], BF16, tag="db", name="db")
                        nc.vector.tensor_mul(
                            out=db[:].rearrange("p (n f) -> p n f", n=NS),
                            in0=_free_repeat(dx[:], NS),
                            in1=B_all[:].rearrange("p (n f) -> p n f", n=NS))
                        h_all = scp.tile([128, NS * TC], BF16, tag="h", name="h_all")
                        for n in range(NS):
                            nc.vector.tensor_tensor_scan(
                                out=h_all[:, n * TC:(n + 1) * TC],
                                data0=da[:, n * TC:(n + 1) * TC],
                                data1=db[:, n * TC:(n + 1) * TC],
                                initial=(0.0 if t == 0 else hcol[j][:, n:n + 1]),
                                op0=ALU.mult, op1=ALU.add)
                        if t + 1 < NT:
                            nc.vector.tensor_copy(
                                out=hcol[j],
                                in_=h_all[:].rearrange("p (n f) -> p n f", n=NS)[:, :, TC - 1])
                        tmp = scp.tile([128, NS * TC], BF16, tag="tmp", name="tmp")
                        nc.vector.tensor_mul(out=tmp, in0=h_all, in1=C_all)
                        w = NS * TC // 2
                        while w > TC:
                            nc.vector.tensor_add(out=tmp[:, 0:w], in0=tmp[:, 0:w],
                                                 in1=tmp[:, w:2 * w])
                            w //= 2
                        nc.vector.tensor_add(out=y_acc[j][:, tsl], in0=tmp[:, 0:TC],
                                             in1=tmp[:, TC:2 * TC])

            # ---- stage 8: D-skip + gate; stage 9: out_proj ----
            with tc.tile_pool(name="outp_pool", bufs=1) as opool:
                y2 = [opool.tile([128, L], BF16, name=f"y2_{j}") for j in range(DBH)]
                for j in range(DBH):
                    nc.vector.scalar_tensor_tensor(
                        out=y2[j], in0=xcb[j], scalar=d_t[:, j:j + 1], in1=y_acc[j],
                        op0=ALU.mult, op1=ALU.add)
                    nc.vector.tensor_mul(out=y2[j], in0=y2[j], in1=zs[j])
                wout_t = [opool.tile([128, DM], BF16, name=f"wo{k}") for k in range(DBH)]
                for k in range(DBH):
                    nc.sync.dma_start(out=wout_t[k], in_=w_out.ap()[k * 128:(k + 1) * 128, :])
                for mj in range(KM):
                    ot = opool.tile([128, L], F32, tag="ot", name="ot")
                    for f in range(2):
                        pm = psA.tile([128, 512], F32, tag="ps", name="ps")
                        for k in range(DBH):
                            nc.tensor.matmul(
                                out=pm, lhsT=wout_t[k][:, mj * 128:(mj + 1) * 128],
                                rhs=y2[k][:, f * 512:(f + 1) * 512],
                                start=(k == 0), stop=(k == DBH - 1))
                        nc.any.tensor_copy(out=ot[:, f * 512:(f + 1) * 512], in_=pm)
                    nc.sync.dma_start(out=outp.ap()[mj * 128:(mj + 1) * 128, :], in_=ot)

    nc.compile()
    return nc


_NC_CACHE = None


def _get_nc():
    global _NC_CACHE
    if _NC_CACHE is None:
        _NC_CACHE = build_nc()
    return _NC_CACHE


def _prep_core(x, ln_g, ln_b, p, h):
    """Build the in_map for one core. p = params dict for this direction,
    h = d_inner half index. x is already time-flipped for bwd cores."""
    lo, hi = h * DH, (h + 1) * DH
    # channel order: my half first, then the other half
    ch = np.concatenate([np.arange(lo, hi), np.arange((1 - h) * DH, (2 - h) * DH)])
    in_w, conv_w, conv_b = p["in_w"], p["conv_w"], p["conv_b"]
    xproj_w, dt_w, dt_b = p["xproj_w"], p["dt_w"], p["dt_b"]
    A_log, Dp, out_w = p["A_log"], p["D"], p["out_w"]

    Wg = in_w * ln_g[None, :]                       # (2*DI, DM)
    bz = in_w @ ln_b                                # (2*DI,)
    rows = np.concatenate([ch, DI + np.arange(lo, hi)])
    w_xz = np.ascontiguousarray(Wg[rows].T.astype(ml_dtypes.bfloat16))  # (DM, 2304)
    b_xz = np.ascontiguousarray(bz[rows].astype(np.float32)[:, None])
    w_cv = np.ascontiguousarray(conv_w[ch].astype(np.float32))          # (DI, 4)
    b_cv = np.ascontiguousarray(conv_b[ch].astype(np.float32)[:, None])
    # xproj output channels: [dt(48), 16 dummy rows, B(16), C(16)] so dt starts at
    # partition 0 and B/C start at the 64-aligned partition 64.
    w_xp96 = np.zeros((DI, 96), np.float32)
    w_xp96[:, 0:DT_RANK] = xproj_w.T[ch][:, 0:DT_RANK]
    w_xp96[:, 64:96] = xproj_w.T[ch][:, DT_RANK:80]
    w_xp = np.ascontiguousarray(w_xp96.astype(ml_dtypes.bfloat16))  # (DI, 96)
    w_dt = np.ascontiguousarray(dt_w[lo:hi].T.astype(ml_dtypes.bfloat16))  # (48, DH)
    b_dt = np.ascontiguousarray(dt_b[lo:hi].astype(np.float32)[:, None])
    a_h = np.ascontiguousarray((-np.exp(A_log[lo:hi])).astype(np.float32))
    d_h = np.ascontiguousarray(Dp[lo:hi].astype(np.float32)[:, None])
    w_out = np.ascontiguousarray(out_w[:, lo:hi].T.astype(ml_dtypes.bfloat16))
    return {
        "xin": np.ascontiguousarray(x.astype(np.float32)),
        "w_xz": w_xz, "b_xz": b_xz, "w_cv": w_cv, "b_cv": b_cv,
        "w_xp": w_xp, "w_dt": w_dt, "b_dt": b_dt, "a_h": a_h, "d_h": d_h,
        "w_out": w_out,
    }


def kernel(**inputs):
    x = np.asarray(inputs["x"], np.float32)          # (2, 1024, 768)
    ln_g = np.asarray(inputs["ln_g"], np.float32)
    ln_b = np.asarray(inputs["ln_b"], np.float32)
    params = {}
    for pref in ("f_", "b_"):
        params[pref] = {k: np.asarray(inputs[pref + k]) for k in
                        ("in_w", "conv_w", "conv_b", "xproj_w", "dt_w", "dt_b",
                         "A_log", "D", "out_w")}
    in_maps = []
    for c in range(N_CORES):
        b, d, h = c // 4, (c % 4) // 2, c % 2
        xb = x[b] if d == 0 else x[b, ::-1]
        in_maps.append(_prep_core(xb, ln_g, ln_b, params["f_" if d == 0 else "b_"], h))

    nc = _get_nc()
    res = bass_utils.run_bass_kernel_spmd(nc, in_maps, core_ids=list(range(N_CORES)))
    outs = [res.results[c]["outp"] for c in range(N_CORES)]   # each (768, 1024)

    out = np.empty_like(x)
    for b in range(2):
        fwd = (outs[b * 4 + 0] + outs[b * 4 + 1]).T            # (1024, 768)
        bwd = (outs[b * 4 + 2] + outs[b * 4 + 3]).T[::-1]
        out[b] = x[b] + fwd + bwd
    return out
